# revision 5
# baseline (speedup 1.0000x reference)
"""Trainium2 Bass kernel for nn_Attend: softmax(q@k^T * scale + bias) @ v.

Shapes (full problem):
  q:         [B=2, H=8, S=2048, D=64] fp32
  k, v:      [B=2, S=2048, D=64]      fp32 (shared across heads)
  mask:      [B=2, S=2048] bool       (all ones in practice)
  attn_bias: [B=2, H=8, S=2048, S=2048] fp32
  out:       [B=2, H=8, S=2048, D=64] fp32
  out = softmax(q@k^T/8 + bias) @ v

Sharding: 16 (b,h) pairs over 8 cores -> 2 heads per core, k/v replicated
per-b (4 cores share each b).

Per-core algorithm (fp16 compute, fp32 PSUM accumulation):
  - bias streams in as fp16 directly via SWDGE cast-DMA (gpsimd ring
    converts f32->f16 inline), 1MB-of-HBM quarters, j-major so the first
    j-tiles' bias lands first; prefetch runs 2 chunks deep (bufs=3).
  - k is pre-scaled by 1/sqrt(D) (exact: 1/8) so q/v load as pure
    cast-DMAs with no DVE pass.
  - kT, qT built once via PE transposes into [128(zero-padded d), S] fp16.
    K padded to 128 so every stationary operand is full-height (FWL path).
  - S^T[j, i] per (head, 512-i-chunk, 128-j-tile): matmul(kT_tile,
    qT_chunk) into PSUM fp32; bias added by 4 matmuls using the NATURAL
    [i', j] fp16 bias block as the STATIONARY operand and a 128x128
    identity as the moving operand (out += bias_blk.T @ I) accumulating
    into the same PSUM bank.
  - P^T = exp(S^T - 2) via ScalarE, PSUM -> SBUF fp16, 1024 cols per
    instruction (softmax is shift-invariant; -2 keeps exp in fp16 range).
  - out^T = accumulated over j-tiles: matmul(v_aug, P^T), v_aug carrying
    a ones-column -> row 64 of out^T is the softmax denominator. PV
    matmuls run one j-pair behind the exp; the epilogue of chunk c runs
    inside chunk c+1 (software-pipelined so PE never waits).
  - Epilogue: 4 PE transposes of stride-4 column slices of out^T so the
    result tile is [p, 4, 64] with partition p holding rows 4p..4p+3
    (1KB-contiguous out-DMA descriptors); reciprocal + scale; out-DMA on
    the ScalarE HWDGE ring.
"""

import sys

sys.path.insert(0, "/opt/trn_rl_repo")

from contextlib import ExitStack

import numpy as np

B, H, S, D = 2, 8, 2048, 64
NH = 2          # heads per core
N_CORES = 8
IC = S // 512   # i-chunks per head
JT = S // 128   # j-tiles
JP = JT // 2    # j-tile pairs
NCHUNK = NH * IC

_cache = {}


def _build():
    import concourse.bacc as bacc
    import concourse.tile as tile
    from concourse import masks, mybir

    f32 = mybir.dt.float32
    f16 = mybir.dt.float16
    Exp = mybir.ActivationFunctionType.Exp

    nc = bacc.Bacc("TRN2", target_bir_lowering=False, debug=False,
                   num_devices=N_CORES)
    q_ap = nc.dram_tensor("q", [NH, S, D], f32, kind="ExternalInput").ap()
    k_ap = nc.dram_tensor("k", [S, D], f32, kind="ExternalInput").ap()
    v_ap = nc.dram_tensor("v", [S, D], f32, kind="ExternalInput").ap()
    bias_ap = nc.dram_tensor("bias", [NH, S, S], f32, kind="ExternalInput").ap()
    out_ap = nc.dram_tensor("out", [NH, S, D], f32, kind="ExternalOutput").ap()

    with tile.TileContext(nc) as tc, ExitStack() as ctx:
        const_pool = ctx.enter_context(tc.tile_pool(name="const", bufs=1))
        prep_sb = ctx.enter_context(tc.tile_pool(name="prep_sb", bufs=1))
        small_ps = ctx.enter_context(
            tc.tile_pool(name="small_ps", bufs=2, space="PSUM"))
        bias_pool = ctx.enter_context(tc.tile_pool(name="bias", bufs=3))
        st_pool = ctx.enter_context(
            tc.tile_pool(name="st", bufs=2, space="PSUM"))
        pt_pool = ctx.enter_context(tc.tile_pool(name="pt", bufs=3))
        ov_pool = ctx.enter_context(
            tc.tile_pool(name="ov", bufs=2, space="PSUM"))
        epi_sb = ctx.enter_context(tc.tile_pool(name="epi_sb", bufs=2))

        # ---- tiles referenced by the DMA front (allocated first)
        kT = const_pool.tile([128, S], f16)
        qT = const_pool.tile([128, NH * S], f16)
        v_aug = const_pool.tile([128, JT * 65], f16)
        k_f = prep_sb.tile([128, JT, 64], f32, tag="kf", name="k_f")
        q16 = prep_sb.tile([128, NH, JT, 64], f16, tag="q16", name="q16")
        v_f = prep_sb.tile([128, JT, 64], f32, tag="vf", name="v_f")

        # bias tiles: [128, quarter(j), s, 512(j)] fp16, cast during DMA
        def new_bias_t(idx):
            return bias_pool.tile([128, 4, 4, 512], f16, tag="biast",
                                  name=f"bias_t{idx}")

        def dma_bias_quarter(bias_t, idx, qr):
            h, c = divmod(idx, IC)
            bsrc = bias_ap[h, c * 512:(c + 1) * 512,
                           qr * 512:(qr + 1) * 512].rearrange(
                "(s p) j -> p s j", p=128)
            nc.gpsimd.dma_start(bias_t[:, qr], bsrc)

        # bias slice for (s, jt): quarter = jt//4, col = (jt%4)*128
        def bias_slice(bias_t, s, jt):
            qr, jc = divmod(jt, 4)
            return bias_t[:, qr, s, jc * 128:(jc + 1) * 128]

        # ---- identities first: make_identity runs on the gpsimd queue and
        # must precede the SWDGE descriptor-generation calls, or the prep
        # transposes stall ~14us behind them.
        ident = const_pool.tile([128, 128], f16)
        masks.make_identity(nc, ident[:])
        shift = const_pool.tile([128, 1], f32)
        nc.vector.memset(shift[:], -2.0)
        warm = const_pool.tile([128, 1], f16)
        nc.scalar.activation(warm[:], shift[:], Exp)  # preload exp tables

        # ---- DMA front, in first-use order.
        # k/v on the sync HWDGE ring; q/bias as SWDGE cast-DMAs (f32->f16).
        kr = k_ap.rearrange("(t p) d -> p t d", p=128)
        q0r = q_ap[0].rearrange("(t p) d -> p t d", p=128)
        nc.sync.dma_start(k_f[:, 0:4], kr[:, 0:4])
        nc.gpsimd.dma_start(q16[:, 0, 0:4], q0r[:, 0:4])
        bias_t0 = new_bias_t(0)
        dma_bias_quarter(bias_t0, 0, 0)
        nc.sync.dma_start(v_f[:], v_ap.rearrange("(t p) d -> p t d", p=128))
        for g in range(1, JT // 4):
            nc.sync.dma_start(k_f[:, g * 4:(g + 1) * 4], kr[:, g * 4:(g + 1) * 4])
        dma_bias_quarter(bias_t0, 0, 1)
        dma_bias_quarter(bias_t0, 0, 2)
        dma_bias_quarter(bias_t0, 0, 3)
        # ident32 gates only the first epilogue (~30us in); emit it after
        # chunk 0's bias but ahead of the lower-urgency SWDGE loads.
        ident32 = const_pool.tile([128, 128], f32)
        masks.make_identity(nc, ident32[:])
        for g in range(1, JT // 4):
            nc.gpsimd.dma_start(q16[:, 0, g * 4:(g + 1) * 4],
                                q0r[:, g * 4:(g + 1) * 4])
        for h in range(1, NH):
            nc.gpsimd.dma_start(
                q16[:, h], q_ap[h].rearrange("(t p) d -> p t d", p=128))
        bias_t1 = new_bias_t(1)
        for qr in range(4):
            dma_bias_quarter(bias_t1, 1, qr)

        nc.vector.memset(kT[64:128, :], 0.0)
        nc.vector.memset(qT[64:128, :], 0.0)
        nc.vector.memset(v_aug[:], 1.0)

        k16 = prep_sb.tile([128, JT, 64], f16, tag="k16", name="k16")

        def transpose_group(src16, g, dst, dst_off):
            p = small_ps.tile([64, 512], f16, tag="sm",
                              name=f"tp_{dst_off}_{g}")
            for u in range(4):
                nc.tensor.matmul(p[:, u * 128:(u + 1) * 128],
                                 src16[:, g * 4 + u], ident[:],
                                 is_transpose=True, start=True, stop=True)
            nc.vector.tensor_copy(
                dst[0:64, dst_off + g * 512: dst_off + (g + 1) * 512], p[:])

        # k: scale by 1/sqrt(D) (exact power of two) during f32->f16
        for g in range(JT // 4):
            nc.vector.tensor_scalar_mul(
                k16[:, g * 4:(g + 1) * 4], k_f[:, g * 4:(g + 1) * 4],
                float(D) ** -0.5)
            transpose_group(k16, g, kT, 0)
        for g in range(JT // 4):
            transpose_group(q16[:, 0], g, qT, 0)
        for jt in range(JT):
            nc.scalar.copy(v_aug[:, jt * 65:jt * 65 + 64], v_f[:, jt])

        # ---- main loop; epilogue of chunk c-1 is emitted inside chunk c
        bias_tiles = {0: bias_t0, 1: bias_t1}
        epi_state = None   # (ov, h, c)

        def emit_epilogue(state):
            ov, eh, ec = state
            ovs = epi_sb.tile([65, 512], f32, tag="ovs")
            nc.vector.tensor_copy(ovs[:], ov[:])
            res = epi_sb.tile([128, 4, 64], f32, tag="res")
            for s in range(4):
                tp = small_ps.tile([128, 65], f32, tag="sm")
                nc.tensor.matmul(tp[:], ovs[:, s::4],
                                 ident32[:65, :65], is_transpose=True,
                                 start=True, stop=True)
                rec = epi_sb.tile([128, 1], f32, tag="rec")
                nc.vector.reciprocal(rec[:], tp[:, 64:65])
                nc.vector.tensor_scalar_mul(res[:, s, :], tp[:, 0:64], rec[:])
            nc.sync.dma_start(
                out_ap[eh, ec * 512:(ec + 1) * 512, :].rearrange(
                    "(p s) d -> p s d", p=128), res[:])

        for idx in range(NCHUNK):
            h, c = divmod(idx, IC)
            bias_t = bias_tiles.pop(idx)
            # prefetch DMA two chunks ahead (SWDGE cast ring)
            if idx + 2 < NCHUNK:
                bias_tiles[idx + 2] = new_bias_t(idx + 2)
                for qr in range(4):
                    dma_bias_quarter(bias_tiles[idx + 2], idx + 2, qr)
            ov = ov_pool.tile([65, 512], f32)
            pt_q = []
            for p in range(JP):
                st = st_pool.tile([128, 1024], f32)
                for u in range(2):
                    jt = 2 * p + u
                    nc.tensor.matmul(
                        st[:, u * 512:(u + 1) * 512],
                        kT[:, jt * 128:(jt + 1) * 128],
                        qT[:, h * S + c * 512: h * S + (c + 1) * 512],
                        start=True, stop=False, skip_group_check=True)
                    for s in range(4):
                        nc.tensor.matmul(
                            st[:, u * 512 + s * 128: u * 512 + (s + 1) * 128],
                            bias_slice(bias_t, s, jt),
                            ident[:], start=False, stop=(s == 3),
                            skip_group_check=True)
                if len(pt_q) >= 2:
                    pp, ppt = pt_q.pop(0)
                    for u in range(2):
                        jt = 2 * pp + u
                        nc.tensor.matmul(
                            ov[:], v_aug[:, jt * 65: jt * 65 + 65],
                            ppt[:, u * 512:(u + 1) * 512],
                            start=(jt == 0), stop=False,
                            skip_group_check=True)
                pt = pt_pool.tile([128, 1024], f16)
                nc.scalar.activation(pt[:], st[:], Exp, bias=shift[:])
                pt_q.append((p, pt))
                if p == 0:
                    # previous chunk's epilogue (PE ops land after pair 0)
                    if epi_state is not None:
                        emit_epilogue(epi_state)
                    if idx == 1:
                        # deferred prep of head 1 (q_h1 DMA lands by now)
                        for g in range(JT // 4):
                            transpose_group(q16[:, 1], g, qT, S)
            while pt_q:
                pp, ppt = pt_q.pop(0)
                for u in range(2):
                    jt = 2 * pp + u
                    nc.tensor.matmul(
                        ov[:], v_aug[:, jt * 65: jt * 65 + 65],
                        ppt[:, u * 512:(u + 1) * 512],
                        start=(jt == 0), stop=(jt == JT - 1),
                        skip_group_check=True)
            epi_state = (ov, h, c)

        emit_epilogue(epi_state)

    nc.compile()
    return nc


def kernel(q, k, v, mask, attn_bias):
    from concourse.bass_utils import run_bass_kernel_spmd

    q = np.ascontiguousarray(np.asarray(q, dtype=np.float32))
    k = np.ascontiguousarray(np.asarray(k, dtype=np.float32))
    v = np.ascontiguousarray(np.asarray(v, dtype=np.float32))
    mask = np.asarray(mask)
    attn_bias = np.asarray(attn_bias, dtype=np.float32)

    if not mask.all():
        attn_bias = np.where(mask[:, None, None, :], attn_bias,
                             np.float32(-3.0e38)).astype(np.float32)

    if "nc" not in _cache:
        _cache["nc"] = _build()
    nc = _cache["nc"]

    in_maps = []
    for c in range(N_CORES):
        b = c // 4
        h0 = NH * (c % 4)
        in_maps.append({
            "q": np.ascontiguousarray(q[b, h0:h0 + NH]),
            "k": k[b],
            "v": v[b],
            "bias": np.ascontiguousarray(attn_bias[b, h0:h0 + NH]),
        })
    res = run_bass_kernel_spmd(nc, in_maps, core_ids=list(range(N_CORES)))
    out = np.empty((B, H, S, D), dtype=np.float32)
    for c in range(N_CORES):
        b = c // 4
        h0 = NH * (c % 4)
        out[c // 4, h0:h0 + NH] = res.results[c]["out"]
    return out


# revision 6
# speedup vs baseline: 1.0104x; 1.0104x over previous
"""Trainium2 Bass kernel for nn_Attend: softmax(q@k^T * scale + bias) @ v.

Shapes (full problem):
  q:         [B=2, H=8, S=2048, D=64] fp32
  k, v:      [B=2, S=2048, D=64]      fp32 (shared across heads)
  mask:      [B=2, S=2048] bool       (all ones in practice)
  attn_bias: [B=2, H=8, S=2048, S=2048] fp32
  out:       [B=2, H=8, S=2048, D=64] fp32
  out = softmax(q@k^T/8 + bias) @ v

Sharding: 16 (b,h) pairs over 8 cores -> 2 heads per core, k/v replicated
per-b (4 cores share each b).

All inputs are cast to fp16 on the HOST (the kernel computes in fp16 with
fp32 PSUM accumulation anyway, so the numerics are identical to casting
on-chip) which halves HBM read traffic -- the dominant cost at full fp32.

Per-core algorithm:
  - bias streams over the sync HWDGE ring in 0.5MB quarters, j-major so
    the first j-tiles' bias lands first; prefetch runs 2 chunks deep.
  - k is pre-scaled by 1/sqrt(D) (exact: 1/8); q loads untouched.
  - kT, qT built once via PE transposes into [128(zero-padded d), S] fp16.
    K padded to 128 so every stationary operand is full-height (FWL path).
  - S^T[j, i] per (head, 512-i-chunk, 128-j-tile): matmul(kT_tile,
    qT_chunk) into PSUM fp32; bias added by 4 matmuls using the NATURAL
    [i', j] fp16 bias block as the STATIONARY operand and a 128x128
    identity as the moving operand (out += bias_blk.T @ I) accumulating
    into the same PSUM bank.
  - P^T = exp(S^T - 2) via ScalarE, PSUM -> SBUF fp16, 1024 cols per
    instruction (softmax is shift-invariant; -2 keeps exp in fp16 range).
  - out^T = accumulated over j-tiles: matmul(v_aug, P^T), v_aug carrying
    a ones-column -> row 64 of out^T is the softmax denominator. PV
    matmuls run one j-pair behind the exp; the epilogue of chunk c runs
    inside chunk c+1 (software-pipelined so PE never waits).
  - Epilogue: 4 PE transposes of stride-4 column slices of out^T so the
    result rows land as [p, 4, 64] with partition p holding rows 4p..4p+3;
    the out-DMA then uses a flat 2D [128, 256] AP (1KB contiguous per
    partition) so HWDGE descriptor generation is cheap.
  - ident16 on gpsimd first; qT pad also on gpsimd (DVE handles kT pad,
    the ones-column strided memset and k scaling) so no engine's prologue
    chain gates the first QK matmul.
"""

import sys

sys.path.insert(0, "/opt/trn_rl_repo")

from contextlib import ExitStack

import numpy as np

B, H, S, D = 2, 8, 2048, 64
NH = 2          # heads per core
N_CORES = 8
IC = S // 512   # i-chunks per head
JT = S // 128   # j-tiles
JP = JT // 2    # j-tile pairs
NCHUNK = NH * IC

_cache = {}


def _build():
    import concourse.bacc as bacc
    import concourse.tile as tile
    from concourse import masks, mybir

    f32 = mybir.dt.float32
    f16 = mybir.dt.float16
    Exp = mybir.ActivationFunctionType.Exp

    nc = bacc.Bacc("TRN2", target_bir_lowering=False, debug=False,
                   num_devices=N_CORES)
    q_ap = nc.dram_tensor("q", [NH, S, D], f16, kind="ExternalInput").ap()
    k_ap = nc.dram_tensor("k", [S, D], f16, kind="ExternalInput").ap()
    v_ap = nc.dram_tensor("v", [S, D], f16, kind="ExternalInput").ap()
    bias_ap = nc.dram_tensor("bias", [NH, S, S], f16, kind="ExternalInput").ap()
    out_ap = nc.dram_tensor("out", [NH, S, D], f32, kind="ExternalOutput").ap()

    with tile.TileContext(nc) as tc, ExitStack() as ctx:
        const_pool = ctx.enter_context(tc.tile_pool(name="const", bufs=1))
        prep_sb = ctx.enter_context(tc.tile_pool(name="prep_sb", bufs=1))
        small_ps = ctx.enter_context(
            tc.tile_pool(name="small_ps", bufs=2, space="PSUM"))
        bias_pool = ctx.enter_context(tc.tile_pool(name="bias", bufs=3))
        st_pool = ctx.enter_context(
            tc.tile_pool(name="st", bufs=2, space="PSUM"))
        pt_pool = ctx.enter_context(tc.tile_pool(name="pt", bufs=3))
        ov_pool = ctx.enter_context(
            tc.tile_pool(name="ov", bufs=2, space="PSUM"))
        epi_sb = ctx.enter_context(tc.tile_pool(name="epi_sb", bufs=2))

        # ---- tiles referenced early
        kT = const_pool.tile([128, S], f16)
        qT = const_pool.tile([128, NH * S], f16)
        v_aug = const_pool.tile([128, JT * 65], f16)
        k_in = prep_sb.tile([128, JT, 64], f16, tag="kin", name="k_in")
        q16 = prep_sb.tile([128, NH, JT, 64], f16, tag="q16", name="q16")
        v16 = prep_sb.tile([128, JT, 64], f16, tag="v16", name="v16")

        # ident16 first on the gpsimd queue: it gates the prep transposes
        ident = const_pool.tile([128, 128], f16)
        masks.make_identity(nc, ident[:])
        shift = const_pool.tile([128, 1], f32)
        nc.vector.memset(shift[:], -2.0)
        warm = const_pool.tile([128, 1], f16)
        nc.scalar.activation(warm[:], shift[:], Exp)  # preload exp tables

        # bias tiles: [128, quarter(j), s, 512(j)] fp16
        def new_bias_t(idx):
            return bias_pool.tile([128, 4, 4, 512], f16, tag="biast",
                                  name=f"bias_t{idx}")

        def dma_bias_quarter(bias_t, idx, qr):
            h, c = divmod(idx, IC)
            bsrc = bias_ap[h, c * 512:(c + 1) * 512,
                           qr * 512:(qr + 1) * 512].rearrange(
                "(s p) j -> p s j", p=128)
            nc.sync.dma_start(bias_t[:, qr], bsrc)

        # bias slice for (s, jt): quarter = jt//4, col = (jt%4)*128
        def bias_slice(bias_t, s, jt):
            qr, jc = divmod(jt, 4)
            return bias_t[:, qr, s, jc * 128:(jc + 1) * 128]

        # ---- DMA front. bias alone on the sync ring; k/q/v on the
        # scalar ring (done before the exp stream occupies its sequencer).
        kr = k_ap.rearrange("(t p) d -> p t d", p=128)
        q0r = q_ap[0].rearrange("(t p) d -> p t d", p=128)
        nc.scalar.dma_start(k_in[:, 0:4], kr[:, 0:4])
        nc.scalar.dma_start(q16[:, 0, 0:4], q0r[:, 0:4])
        bias_t0 = new_bias_t(0)
        dma_bias_quarter(bias_t0, 0, 0)
        nc.scalar.dma_start(v16[:], v_ap.rearrange("(t p) d -> p t d", p=128))
        nc.scalar.dma_start(k_in[:, 4:16], kr[:, 4:16])
        dma_bias_quarter(bias_t0, 0, 1)
        nc.scalar.dma_start(q16[:, 0, 4:16], q0r[:, 4:16])
        dma_bias_quarter(bias_t0, 0, 2)
        dma_bias_quarter(bias_t0, 0, 3)
        for h in range(1, NH):
            nc.scalar.dma_start(
                q16[:, h], q_ap[h].rearrange("(t p) d -> p t d", p=128))
        bias_t1 = new_bias_t(1)
        for qr in range(4):
            dma_bias_quarter(bias_t1, 1, qr)

        # pads: qT on gpsimd (free after ident), kT + ones-cols on DVE
        nc.gpsimd.memset(qT[64:128, :], 0.0)
        ident32 = const_pool.tile([128, 128], f32)
        masks.make_identity(nc, ident32[:])
        nc.vector.memset(kT[64:128, :], 0.0)
        nc.vector.memset(
            v_aug.rearrange("p (t c) -> p t c", c=65)[:, :, 64:65], 1.0)

        k16 = prep_sb.tile([128, JT, 64], f16, tag="k16", name="k16")

        def transpose_group(src16, g, dst, dst_off):
            p = small_ps.tile([64, 512], f16, tag="sm",
                              name=f"tp_{dst_off}_{g}")
            for u in range(4):
                nc.tensor.matmul(p[:, u * 128:(u + 1) * 128],
                                 src16[:, g * 4 + u], ident[:],
                                 is_transpose=True, start=True, stop=True)
            nc.vector.tensor_copy(
                dst[0:64, dst_off + g * 512: dst_off + (g + 1) * 512], p[:])

        # k: scale by 1/sqrt(D) (exact power of two)
        for g in range(JT // 4):
            nc.vector.tensor_scalar_mul(
                k16[:, g * 4:(g + 1) * 4], k_in[:, g * 4:(g + 1) * 4],
                float(D) ** -0.5)
            transpose_group(k16, g, kT, 0)
        for g in range(JT // 4):
            transpose_group(q16[:, 0], g, qT, 0)
        for jt in range(JT):
            nc.scalar.copy(v_aug[:, jt * 65:jt * 65 + 64], v16[:, jt])

        # ---- main loop; epilogue of chunk c-1 is emitted inside chunk c
        bias_tiles = {0: bias_t0, 1: bias_t1}
        epi_state = None   # (ov, h, c)

        def emit_epilogue(state):
            ov, eh, ec = state
            ovs = epi_sb.tile([65, 512], f32, tag="ovs")
            nc.vector.tensor_copy(ovs[:], ov[:])
            res = epi_sb.tile([128, 4, 64], f32, tag="res")
            for s in range(4):
                tp = small_ps.tile([128, 65], f32, tag="sm")
                nc.tensor.matmul(tp[:], ovs[:, s::4],
                                 ident32[:65, :65], is_transpose=True,
                                 start=True, stop=True)
                rec = epi_sb.tile([128, 1], f32, tag="rec")
                nc.vector.reciprocal(rec[:], tp[:, 64:65])
                nc.vector.tensor_scalar_mul(res[:, s, :], tp[:, 0:64], rec[:])
            # rows 4p..4p+3 live on partition p -> flat 2D AP, 1KB
            # contiguous per partition (cheap HWDGE descriptor gen)
            nc.scalar.dma_start(
                out_ap[eh, ec * 512:(ec + 1) * 512, :].rearrange(
                    "(p s) d -> p (s d)", p=128),
                res[:].rearrange("p s d -> p (s d)"))

        for idx in range(NCHUNK):
            h, c = divmod(idx, IC)
            bias_t = bias_tiles.pop(idx)
            # prefetch DMA two chunks ahead
            if idx + 2 < NCHUNK:
                bias_tiles[idx + 2] = new_bias_t(idx + 2)
                for qr in range(4):
                    dma_bias_quarter(bias_tiles[idx + 2], idx + 2, qr)
            ov = ov_pool.tile([65, 512], f32)
            pt_q = []
            for p in range(JP):
                st = st_pool.tile([128, 1024], f32)
                for u in range(2):
                    jt = 2 * p + u
                    nc.tensor.matmul(
                        st[:, u * 512:(u + 1) * 512],
                        kT[:, jt * 128:(jt + 1) * 128],
                        qT[:, h * S + c * 512: h * S + (c + 1) * 512],
                        start=True, stop=False, skip_group_check=True)
                    for s in range(4):
                        nc.tensor.matmul(
                            st[:, u * 512 + s * 128: u * 512 + (s + 1) * 128],
                            bias_slice(bias_t, s, jt),
                            ident[:], start=False, stop=(s == 3),
                            skip_group_check=True)
                if len(pt_q) >= 2:
                    pp, ppt = pt_q.pop(0)
                    for u in range(2):
                        jt = 2 * pp + u
                        nc.tensor.matmul(
                            ov[:], v_aug[:, jt * 65: jt * 65 + 65],
                            ppt[:, u * 512:(u + 1) * 512],
                            start=(jt == 0), stop=False,
                            skip_group_check=True)
                pt = pt_pool.tile([128, 1024], f16)
                nc.scalar.activation(pt[:], st[:], Exp, bias=shift[:])
                pt_q.append((p, pt))
                if p == 0:
                    # previous chunk's epilogue (PE ops land after pair 0)
                    if epi_state is not None:
                        emit_epilogue(epi_state)
                    if idx == 1:
                        # deferred prep of head 1 (q_h1 DMA lands by now)
                        for g in range(JT // 4):
                            transpose_group(q16[:, 1], g, qT, S)
            while pt_q:
                pp, ppt = pt_q.pop(0)
                for u in range(2):
                    jt = 2 * pp + u
                    nc.tensor.matmul(
                        ov[:], v_aug[:, jt * 65: jt * 65 + 65],
                        ppt[:, u * 512:(u + 1) * 512],
                        start=(jt == 0), stop=(jt == JT - 1),
                        skip_group_check=True)
            epi_state = (ov, h, c)

        emit_epilogue(epi_state)

    nc.compile()
    return nc


def kernel(q, k, v, mask, attn_bias):
    from concourse.bass_utils import run_bass_kernel_spmd

    q = np.asarray(q, dtype=np.float32)
    k = np.asarray(k, dtype=np.float32)
    v = np.asarray(v, dtype=np.float32)
    mask = np.asarray(mask)
    attn_bias = np.asarray(attn_bias, dtype=np.float32)

    if not mask.all():
        attn_bias = np.where(mask[:, None, None, :], attn_bias,
                             np.float32(-3.0e38)).astype(np.float32)

    # fp16 on host: identical numerics to the on-chip cast the kernel
    # would otherwise do, at half the HBM traffic.
    q16 = q.astype(np.float16)
    k16 = k.astype(np.float16)
    v16 = v.astype(np.float16)
    bias16 = attn_bias.astype(np.float16)

    if "nc" not in _cache:
        _cache["nc"] = _build()
    nc = _cache["nc"]

    in_maps = []
    for c in range(N_CORES):
        b = c // 4
        h0 = NH * (c % 4)
        in_maps.append({
            "q": np.ascontiguousarray(q16[b, h0:h0 + NH]),
            "k": np.ascontiguousarray(k16[b]),
            "v": np.ascontiguousarray(v16[b]),
            "bias": np.ascontiguousarray(bias16[b, h0:h0 + NH]),
        })
    res = run_bass_kernel_spmd(nc, in_maps, core_ids=list(range(N_CORES)))
    out = np.empty((B, H, S, D), dtype=np.float32)
    for c in range(N_CORES):
        b = c // 4
        h0 = NH * (c % 4)
        out[b, h0:h0 + NH] = res.results[c]["out"]
    return out
